# revision 8
# baseline (speedup 1.0000x reference)
"""Tensor-parallel GQA attention (Llama-3-8B shape, prefill, start_pos=0) on 8
Trainium2 NeuronCores.

Sharding: core i owns kv-head i and q-heads 4i..4i+3 — wq/wk/wv column-shards,
wo row-shard, x replicated.  Each core computes a partial [2048, 4096] output
(its heads pushed through its wo rows); the host sums the 8 partials
(all-reduce equivalent).

Per-core kernel layout choices (all matmuls N=512, fp32r operands):
  - xT [D, S] on device; projections computed with weights as the stationary
    operand, giving qT/kT/vT in [head_dim, seq] layout directly.
  - RoPE applied in [head_dim, seq] layout; the rotate-half partition swap is
    two SBUF->SBUF DMAs; sin tables are sign-folded on the host, and the
    1/sqrt(hd) score scale is folded into q's sin/cos tables.
  - Scores computed transposed, ST[j, i] = kT.T @ qT, so exp needs no
    transposes and PV consumes Pexp^T directly (lhsT = v tile [j, d],
    rhs = Pexp^T [j, i] -> outT [d, i] accumulated over j-tiles in PSUM).
  - No max-subtraction in softmax (scores bounded, |S| ~ 10); row sums come
    from an all-ones stationary matmul accumulated alongside PV (broadcast
    across partitions), so normalization is one reciprocal + one multiply,
    fused with the PV eviction.  outT overwrites qT storage (dead by then).
  - Causal masking: only j-tiles with j <= chunk max are computed; the 4
    diagonal tiles per (head, i-chunk) get affine_select(fill=0) after exp.
  - wo matmul with lhsT = normalized outT slices, accumulated over the 4
    heads in PSUM; eviction is a plain copy, DMA'd to the partial output.
"""

import math
from contextlib import ExitStack

import numpy as np

import concourse.bass as bass
import concourse.tile as tile
from concourse import bacc, mybir
from concourse.bass_utils import run_bass_kernel_spmd

# ---- problem shape (hardcoded per contract) ----
S = 2048           # seq len
D = 4096           # model dim
HD = 128           # head dim
N_CORES = 8
NQH = 4            # q heads per core
QCOLS = NQH * HD   # 512 wq columns per core
SC_N = 4           # seq chunks of 512
KT_N = D // 128    # 32 contraction tiles
JT_N = S // 128    # 16 key tiles
ECH_N = D // 512   # 8 output column chunks

F32 = mybir.dt.float32
F32R = mybir.dt.float32r

_BUILD_CACHE: dict = {}


def _rope_apply(nc, pools, dst_slice, ps, cos_t, sin_t):
    """dst = ps * cos + swap64(ps) * sin   (all [128, 512])."""
    qc = pools["rope_tmp"].tile([128, 512], F32, tag="rope_qc", name="rqc")
    nc.vector.tensor_copy(qc[:], ps)
    qs = pools["rope_tmp"].tile([128, 512], F32, tag="rope_qs", name="rqs")
    nc.sync.dma_start(qs[0:64, :], qc[64:128, :])
    nc.sync.dma_start(qs[64:128, :], qc[0:64, :])
    tc_ = pools["rope_tmp"].tile([128, 512], F32, tag="rope_tc", name="rtc")
    nc.vector.tensor_mul(tc_[:], qc[:], cos_t)
    ts_ = pools["rope_tmp"].tile([128, 512], F32, tag="rope_ts", name="rts")
    nc.vector.tensor_mul(ts_[:], qs[:], sin_t)
    nc.vector.tensor_add(dst_slice, tc_[:], ts_[:])


def build_nc(causal: bool = True):
    nc = bacc.Bacc(
        "TRN2", target_bir_lowering=False, debug=False, num_devices=N_CORES
    )
    dram = {}
    for name, shape, dt in [
        ("xT", [D, S], F32R),
        ("wq", [D, QCOLS], F32R),
        ("wk", [D, HD], F32R),
        ("wv", [D, HD], F32R),
        ("wo", [QCOLS, D], F32R),
        ("cosq", [HD, S], F32),
        ("sinq", [HD, S], F32),
        ("cosk", [HD, S], F32),
        ("sink", [HD, S], F32),
        ("ones", [128, 128], F32R),
        ("ident", [128, 128], F32R),
    ]:
        dram[name] = nc.dram_tensor(name, shape, dt, kind="ExternalInput").ap()
    out = nc.dram_tensor("out", [S, D], F32, kind="ExternalOutput").ap()

    with tile.TileContext(nc) as tc, ExitStack() as ctx:
        def pool(name, bufs, space="SBUF"):
            return ctx.enter_context(tc.tile_pool(name=name, bufs=bufs, space=space))

        pools = {
            "w": pool("w", 1),
            "wq_s": pool("wq_s", 4),
            "x": pool("x", 4),
            "rope_in": pool("rope_in", 2),
            "rope_tmp": pool("rope_tmp", 2),
            "persist": pool("persist", 1),
            "pexp": pool("pexp", 4),
            "recip": pool("recip", 2),
            "wo": pool("wo", 2),
            "outsb": pool("outsb", 3),
        }

        # resident small weights (wk/wv); wq is streamed per (sc, kt)
        wk_sb = pools["w"].tile([128, KT_N, HD], F32R, tag="wk", name="wk_sb")
        nc.sync.dma_start(
            wk_sb[:], dram["wk"].rearrange("(a p) m -> p a m", p=128)
        )
        wv_sb = pools["w"].tile([128, KT_N, HD], F32R, tag="wv", name="wv_sb")
        nc.sync.dma_start(
            wv_sb[:], dram["wv"].rearrange("(a p) m -> p a m", p=128)
        )

        # persistent activations
        kT_sb = pools["persist"].tile([128, S], F32R, tag="kT", name="kT_sb")
        # qT doubles as outT: B(h, ic) consumes qT[h, ic] then writes the
        # normalized attention output into the same slice.
        qT_sb = pools["persist"].tile([128, NQH, S], F32R, tag="qT", name="qT_sb")
        v_sb = pools["persist"].tile([128, JT_N, HD], F32R, tag="v", name="v_sb")
        ones_sb = pools["persist"].tile([128, 128], F32R, tag="ones", name="ones_sb")
        nc.sync.dma_start(ones_sb[:], dram["ones"][:])
        ident_sb = pools["persist"].tile([128, 128], F32R, tag="ident", name="ident_sb")
        nc.sync.dma_start(ident_sb[:], dram["ident"][:])

        # ---- stage A: projections + RoPE, per 512-wide seq chunk ----
        with tc.tile_pool(name="psA", bufs=1, space="PSUM") as psA:
            for sc in range(SC_N):
                ssl = slice(sc * 512, (sc + 1) * 512)
                cq = pools["rope_in"].tile([128, 512], F32, tag="cq", name="cq")
                nc.sync.dma_start(cq[:], dram["cosq"][:, ssl])
                sq = pools["rope_in"].tile([128, 512], F32, tag="sq", name="sq")
                nc.sync.dma_start(sq[:], dram["sinq"][:, ssl])
                ck = pools["rope_in"].tile([128, 512], F32, tag="ck", name="ck")
                nc.sync.dma_start(ck[:], dram["cosk"][:, ssl])
                sk = pools["rope_in"].tile([128, 512], F32, tag="sk", name="sk")
                nc.sync.dma_start(sk[:], dram["sink"][:, ssl])

                ps_q = [
                    psA.tile([128, 512], F32, tag=f"psq{h}", name=f"psq{h}")
                    for h in range(NQH)
                ]
                ps_k = psA.tile([128, 512], F32, tag="psk", name="psk")
                ps_vt = psA.tile([128, 512], F32, tag="psvt", name="psvt")
                for kt in range(KT_N):
                    xt = pools["x"].tile([128, 512], F32R, tag="xt", name="xt")
                    nc.sync.dma_start(
                        xt[:], dram["xT"][kt * 128:(kt + 1) * 128, ssl]
                    )
                    wq_t = pools["wq_s"].tile(
                        [128, QCOLS], F32R, tag="wq_t", name="wq_t"
                    )
                    nc.sync.dma_start(
                        wq_t[:], dram["wq"][kt * 128:(kt + 1) * 128, :]
                    )
                    first, last = kt == 0, kt == KT_N - 1
                    for h in range(NQH):
                        nc.tensor.matmul(
                            ps_q[h][:],
                            wq_t[:, h * 128:(h + 1) * 128],
                            xt[:],
                            start=first,
                            stop=last,
                        )
                    nc.tensor.matmul(
                        ps_k[:], wk_sb[:, kt, :], xt[:], start=first, stop=last
                    )
                    nc.tensor.matmul(
                        ps_vt[:], wv_sb[:, kt, :], xt[:], start=first, stop=last
                    )

                _rope_apply(nc, pools, kT_sb[:, ssl], ps_k[:], ck[:], sk[:])
                # v: evict vT then transpose 128x128 blocks to [j, d]
                vt_f = pools["rope_tmp"].tile(
                    [128, 512], F32R, tag="vt_f", name="vt_f"
                )
                nc.vector.tensor_copy(vt_f[:], ps_vt[:])
                for vi in range(4):
                    ptr = psA.tile([128, 128], F32R, tag="pstr", name="pstr")
                    nc.tensor.transpose(
                        ptr[:], vt_f[:, vi * 128:(vi + 1) * 128], ident_sb[:]
                    )
                    nc.vector.tensor_copy(v_sb[:, sc * 4 + vi, :], ptr[:])
                for h in range(NQH):
                    _rope_apply(
                        nc, pools, qT_sb[:, h, ssl], ps_q[h][:], cq[:], sq[:]
                    )

        # ---- stage B: attention (transposed scores), per (i-chunk, head) ----
        with tc.tile_pool(name="psB", bufs=1, space="PSUM") as psB:
            for ic in range(SC_N):
                isl = slice(ic * 512, (ic + 1) * 512)
                njt = 4 * (ic + 1) if causal else JT_N
                for h in range(NQH):
                    pv = psB.tile([128, 512], F32, tag="pspv", name="pspv")
                    rs = psB.tile([128, 512], F32, tag="psrs", name="psrs")
                    for jt in range(njt):
                        st = psB.tile(
                            [128, 512], F32, tag="psst", name="psst", bufs=2
                        )
                        nc.tensor.matmul(
                            st[:],
                            kT_sb[:, jt * 128:(jt + 1) * 128],
                            qT_sb[:, h, isl],
                            start=True,
                            stop=True,
                        )
                        pe = pools["pexp"].tile([128, 512], F32R, tag="pe", name="pe")
                        nc.scalar.activation(
                            pe[:], st[:], mybir.ActivationFunctionType.Exp
                        )
                        if causal and jt >= 4 * ic:
                            nc.gpsimd.affine_select(
                                out=pe[:],
                                in_=pe[:],
                                pattern=[[1, 512]],
                                compare_op=mybir.AluOpType.is_ge,
                                fill=0.0,
                                base=512 * ic - 128 * jt,
                                channel_multiplier=-1,
                            )
                        first, last = jt == 0, jt == njt - 1
                        nc.tensor.matmul(
                            pv[:], v_sb[:, jt, :], pe[:], start=first, stop=last
                        )
                        nc.tensor.matmul(
                            rs[:], ones_sb[:], pe[:], start=first, stop=last
                        )
                    rc = pools["recip"].tile([128, 512], F32, tag="rc", name="rc")
                    nc.vector.reciprocal(rc[:], rs[:])
                    nc.vector.tensor_mul(qT_sb[:, h, isl], pv[:], rc[:])

            # ---- stage C: wo matmul (outT lives in qT_sb) ----
            for ech in range(ECH_N):
                esl = slice(ech * 512, (ech + 1) * 512)
                woc = pools["wo"].tile([128, NQH, 512], F32R, tag="woc", name="woc")
                nc.sync.dma_start(
                    woc[:], dram["wo"][:, esl].rearrange("(a p) n -> p a n", p=128)
                )
                for it in range(JT_N):
                    pc = psB.tile([128, 512], F32, tag="psc", name="psc", bufs=2)
                    for h in range(NQH):
                        nc.tensor.matmul(
                            pc[:],
                            qT_sb[:, h, it * 128:(it + 1) * 128],
                            woc[:, h, :],
                            start=h == 0,
                            stop=h == NQH - 1,
                        )
                    ob = pools["outsb"].tile([128, 512], F32, tag="ob", name="ob")
                    nc.vector.tensor_copy(ob[:], pc[:])
                    nc.sync.dma_start(
                        out[it * 128:(it + 1) * 128, esl], ob[:]
                    )

    nc.compile()
    return nc


def get_nc(causal: bool = True):
    if causal not in _BUILD_CACHE:
        _BUILD_CACHE[causal] = build_nc(causal)
    return _BUILD_CACHE[causal]


def prep_in_maps(x, sincos, wq, wk, wv, wo):
    """Host-side shard + layout prep. Returns list of per-core input dicts."""
    x = np.asarray(x, np.float32)
    assert x.shape == (1, S, D)
    xT = np.ascontiguousarray(x[0].T)

    sincos = np.asarray(sincos, np.float32)
    sin = sincos[:S, :HD]
    cos = sincos[:S, HD:]
    sinT = np.ascontiguousarray(sin.T)
    cosT = np.ascontiguousarray(cos.T)
    sin_sgn = sinT.copy()
    sin_sgn[:64] = -sinT[:64]
    scale = np.float32(1.0 / math.sqrt(HD))
    cosq, sinq = cosT * scale, sin_sgn * scale
    cosk, sink = cosT, sin_sgn

    wq = np.asarray(wq, np.float32)
    wk = np.asarray(wk, np.float32)
    wv = np.asarray(wv, np.float32)
    wo = np.asarray(wo, np.float32)

    in_maps = []
    for c in range(N_CORES):
        in_maps.append(
            {
                "xT": xT,
                "wq": np.ascontiguousarray(wq[:, c * QCOLS:(c + 1) * QCOLS]),
                "wk": np.ascontiguousarray(wk[:, c * HD:(c + 1) * HD]),
                "wv": np.ascontiguousarray(wv[:, c * HD:(c + 1) * HD]),
                "wo": np.ascontiguousarray(wo[c * QCOLS:(c + 1) * QCOLS, :]),
                "cosq": cosq,
                "sinq": sinq,
                "cosk": cosk,
                "sink": sink,
                "ones": np.ones((128, 128), np.float32),
                "ident": np.eye(128, dtype=np.float32),
            }
        )
    return in_maps


def check_mask(full_causal_mask, start_pos) -> bool:
    """Returns True for causal (tril) mask, False for all-allowed."""
    sp = int(start_pos)
    assert sp == 0, f"kernel specialized for start_pos=0, got {sp}"
    m = np.asarray(full_causal_mask)
    assert m.shape == (1, 1, S, S)
    m = m[0, 0]
    tril = np.tril(np.ones((S, S), dtype=bool))
    if (m == tril).all():
        return True
    if m.all():
        return False
    raise AssertionError("unsupported mask pattern")


def kernel(
    x,
    start_pos,
    sincos,
    full_causal_mask,
    wq,
    wk,
    wv,
    wo,
    cache_k,
    cache_v,
):
    causal = check_mask(full_causal_mask, start_pos)
    # cache_k/cache_v are zero and fully overwritten in the attended region
    # (start_pos=0, seq_len == max_seq_len) — they do not affect the output.
    nc = get_nc(causal)
    in_maps = prep_in_maps(x, sincos, wq, wk, wv, wo)
    res = run_bass_kernel_spmd(nc, in_maps, list(range(N_CORES)))
    acc = res.results[0]["out"].astype(np.float32)
    for c in range(1, N_CORES):
        acc = acc + res.results[c]["out"]
    return acc[np.newaxis]


# revision 9
# speedup vs baseline: 46.5020x; 46.5020x over previous
"""Tensor-parallel GQA attention (Llama-3-8B shape, prefill, start_pos=0) on 8
Trainium2 NeuronCores.

Sharding: core i owns kv-head i and q-heads 4i..4i+3 — wq/wk/wv column-shards,
wo row-shard, x replicated.  Each core computes a partial [2048, 4096] output
(its heads pushed through its wo rows); the host sums the 8 partials
(all-reduce equivalent).

Per-core kernel layout choices (all matmuls N=512, fp32r operands):
  - xT [D, S] on device; projections computed with weights as the stationary
    operand, giving qT/kT/vT in [head_dim, seq] layout directly.
  - RoPE applied in [head_dim, seq] layout; the rotate-half partition swap is
    two SBUF->SBUF DMAs; sin tables are sign-folded on the host, and the
    1/sqrt(hd) score scale is folded into q's sin/cos tables.
  - Scores computed transposed, ST[j, i] = kT.T @ qT, so exp needs no
    transposes and PV consumes Pexp^T directly (lhsT = v tile [j, d],
    rhs = Pexp^T [j, i] -> outT [d, i] accumulated over j-tiles in PSUM).
  - No max-subtraction in softmax (scores bounded, |S| ~ 10); row sums come
    from an all-ones stationary matmul accumulated alongside PV (broadcast
    across partitions), so normalization is one reciprocal + one multiply,
    fused with the PV eviction.  outT overwrites qT storage (dead by then).
  - Causal masking: only j-tiles with j <= chunk max are computed; the 4
    diagonal tiles per (head, i-chunk) get affine_select(fill=0) after exp.
  - wo matmul with lhsT = normalized outT slices, accumulated over the 4
    heads in PSUM; eviction is a plain copy, DMA'd to the partial output.
"""

import math
from contextlib import ExitStack

import numpy as np

import concourse.bass as bass
import concourse.tile as tile
from concourse import bacc, mybir
from concourse.bass_utils import run_bass_kernel_spmd

# ---- problem shape (hardcoded per contract) ----
S = 2048           # seq len
D = 4096           # model dim
HD = 128           # head dim
N_CORES = 8
NQH = 4            # q heads per core
QCOLS = NQH * HD   # 512 wq columns per core
SC_N = 4           # seq chunks of 512
KT_N = D // 128    # 32 contraction tiles
JT_N = S // 128    # 16 key tiles
ECH_N = D // 512   # 8 output column chunks

F32 = mybir.dt.float32
F32R = mybir.dt.float32r

_BUILD_CACHE: dict = {}


def _rope_apply(nc, pools, dst_slice, ps, cos_t, sin_t):
    """dst = ps * cos + swap64(ps) * sin   (all [128, 512])."""
    qc = pools["rope_tmp"].tile([128, 512], F32, tag="rope_qc", name="rqc")
    nc.vector.tensor_copy(qc[:], ps)
    qs = pools["rope_tmp"].tile([128, 512], F32, tag="rope_qs", name="rqs")
    nc.sync.dma_start(qs[0:64, :], qc[64:128, :])
    nc.sync.dma_start(qs[64:128, :], qc[0:64, :])
    tc_ = pools["rope_tmp"].tile([128, 512], F32, tag="rope_tc", name="rtc")
    nc.vector.tensor_mul(tc_[:], qc[:], cos_t)
    ts_ = pools["rope_tmp"].tile([128, 512], F32, tag="rope_ts", name="rts")
    nc.vector.tensor_mul(ts_[:], qs[:], sin_t)
    nc.vector.tensor_add(dst_slice, tc_[:], ts_[:])


def _emit_body(nc, tc, dram, out, causal: bool):
    with ExitStack() as ctx:
        def pool(name, bufs, space="SBUF"):
            return ctx.enter_context(tc.tile_pool(name=name, bufs=bufs, space=space))

        pools = {
            "w": pool("w", 1),
            "wq_s": pool("wq_s", 4),
            "x": pool("x", 4),
            "rope_in": pool("rope_in", 2),
            "rope_tmp": pool("rope_tmp", 2),
            "persist": pool("persist", 1),
            "pexp": pool("pexp", 4),
            "recip": pool("recip", 2),
            "wo": pool("wo", 2),
            "outsb": pool("outsb", 3),
        }

        # resident small weights (wk/wv); wq is streamed per (sc, kt)
        wk_sb = pools["w"].tile([128, KT_N, HD], F32R, tag="wk", name="wk_sb")
        nc.sync.dma_start(
            wk_sb[:], dram["wk"].rearrange("(a p) m -> p a m", p=128)
        )
        wv_sb = pools["w"].tile([128, KT_N, HD], F32R, tag="wv", name="wv_sb")
        nc.sync.dma_start(
            wv_sb[:], dram["wv"].rearrange("(a p) m -> p a m", p=128)
        )

        # persistent activations
        kT_sb = pools["persist"].tile([128, S], F32R, tag="kT", name="kT_sb")
        # qT doubles as outT: B(h, ic) consumes qT[h, ic] then writes the
        # normalized attention output into the same slice.
        qT_sb = pools["persist"].tile([128, NQH, S], F32R, tag="qT", name="qT_sb")
        v_sb = pools["persist"].tile([128, JT_N, HD], F32R, tag="v", name="v_sb")
        ones_sb = pools["persist"].tile([128, 128], F32R, tag="ones", name="ones_sb")
        nc.sync.dma_start(ones_sb[:], dram["ones"][:])
        ident_sb = pools["persist"].tile([128, 128], F32R, tag="ident", name="ident_sb")
        nc.sync.dma_start(ident_sb[:], dram["ident"][:])

        # ---- stage A: projections + RoPE, per 512-wide seq chunk ----
        with tc.tile_pool(name="psA", bufs=1, space="PSUM") as psA:
            for sc in range(SC_N):
                ssl = slice(sc * 512, (sc + 1) * 512)
                cq = pools["rope_in"].tile([128, 512], F32, tag="cq", name="cq")
                nc.sync.dma_start(cq[:], dram["cosq"][:, ssl])
                sq = pools["rope_in"].tile([128, 512], F32, tag="sq", name="sq")
                nc.sync.dma_start(sq[:], dram["sinq"][:, ssl])
                ck = pools["rope_in"].tile([128, 512], F32, tag="ck", name="ck")
                nc.sync.dma_start(ck[:], dram["cosk"][:, ssl])
                sk = pools["rope_in"].tile([128, 512], F32, tag="sk", name="sk")
                nc.sync.dma_start(sk[:], dram["sink"][:, ssl])

                ps_q = [
                    psA.tile([128, 512], F32, tag=f"psq{h}", name=f"psq{h}")
                    for h in range(NQH)
                ]
                ps_k = psA.tile([128, 512], F32, tag="psk", name="psk")
                ps_vt = psA.tile([128, 512], F32, tag="psvt", name="psvt")
                for kt in range(KT_N):
                    xt = pools["x"].tile([128, 512], F32R, tag="xt", name="xt")
                    nc.sync.dma_start(
                        xt[:], dram["xT"][kt * 128:(kt + 1) * 128, ssl]
                    )
                    wq_t = pools["wq_s"].tile(
                        [128, QCOLS], F32R, tag="wq_t", name="wq_t"
                    )
                    nc.sync.dma_start(
                        wq_t[:], dram["wq"][kt * 128:(kt + 1) * 128, :]
                    )
                    first, last = kt == 0, kt == KT_N - 1
                    for h in range(NQH):
                        nc.tensor.matmul(
                            ps_q[h][:],
                            wq_t[:, h * 128:(h + 1) * 128],
                            xt[:],
                            start=first,
                            stop=last,
                        )
                    nc.tensor.matmul(
                        ps_k[:], wk_sb[:, kt, :], xt[:], start=first, stop=last
                    )
                    nc.tensor.matmul(
                        ps_vt[:], wv_sb[:, kt, :], xt[:], start=first, stop=last
                    )

                _rope_apply(nc, pools, kT_sb[:, ssl], ps_k[:], ck[:], sk[:])
                # v: evict vT then transpose 128x128 blocks to [j, d]
                vt_f = pools["rope_tmp"].tile(
                    [128, 512], F32R, tag="vt_f", name="vt_f"
                )
                nc.vector.tensor_copy(vt_f[:], ps_vt[:])
                for vi in range(4):
                    ptr = psA.tile([128, 128], F32R, tag="pstr", name="pstr")
                    nc.tensor.transpose(
                        ptr[:], vt_f[:, vi * 128:(vi + 1) * 128], ident_sb[:]
                    )
                    nc.vector.tensor_copy(v_sb[:, sc * 4 + vi, :], ptr[:])
                for h in range(NQH):
                    _rope_apply(
                        nc, pools, qT_sb[:, h, ssl], ps_q[h][:], cq[:], sq[:]
                    )

        # ---- stage B: attention (transposed scores), per (i-chunk, head) ----
        with tc.tile_pool(name="psB", bufs=1, space="PSUM") as psB:
            for ic in range(SC_N):
                isl = slice(ic * 512, (ic + 1) * 512)
                njt = 4 * (ic + 1) if causal else JT_N
                for h in range(NQH):
                    pv = psB.tile([128, 512], F32, tag="pspv", name="pspv")
                    rs = psB.tile([128, 512], F32, tag="psrs", name="psrs")
                    for jt in range(njt):
                        st = psB.tile(
                            [128, 512], F32, tag="psst", name="psst", bufs=2
                        )
                        nc.tensor.matmul(
                            st[:],
                            kT_sb[:, jt * 128:(jt + 1) * 128],
                            qT_sb[:, h, isl],
                            start=True,
                            stop=True,
                        )
                        pe = pools["pexp"].tile([128, 512], F32R, tag="pe", name="pe")
                        nc.scalar.activation(
                            pe[:], st[:], mybir.ActivationFunctionType.Exp
                        )
                        if causal and jt >= 4 * ic:
                            nc.gpsimd.affine_select(
                                out=pe[:],
                                in_=pe[:],
                                pattern=[[1, 512]],
                                compare_op=mybir.AluOpType.is_ge,
                                fill=0.0,
                                base=512 * ic - 128 * jt,
                                channel_multiplier=-1,
                            )
                        first, last = jt == 0, jt == njt - 1
                        nc.tensor.matmul(
                            pv[:], v_sb[:, jt, :], pe[:], start=first, stop=last
                        )
                        nc.tensor.matmul(
                            rs[:], ones_sb[:], pe[:], start=first, stop=last
                        )
                    rc = pools["recip"].tile([128, 512], F32, tag="rc", name="rc")
                    nc.vector.reciprocal(rc[:], rs[:])
                    nc.vector.tensor_mul(qT_sb[:, h, isl], pv[:], rc[:])

            # ---- stage C: wo matmul (outT lives in qT_sb) ----
            for ech in range(ECH_N):
                esl = slice(ech * 512, (ech + 1) * 512)
                woc = pools["wo"].tile([128, NQH, 512], F32R, tag="woc", name="woc")
                nc.sync.dma_start(
                    woc[:], dram["wo"][:, esl].rearrange("(a p) n -> p a n", p=128)
                )
                for it in range(JT_N):
                    pc = psB.tile([128, 512], F32, tag="psc", name="psc", bufs=2)
                    for h in range(NQH):
                        nc.tensor.matmul(
                            pc[:],
                            qT_sb[:, h, it * 128:(it + 1) * 128],
                            woc[:, h, :],
                            start=h == 0,
                            stop=h == NQH - 1,
                        )
                    ob = pools["outsb"].tile([128, 512], F32, tag="ob", name="ob")
                    nc.vector.tensor_copy(ob[:], pc[:])
                    nc.sync.dma_start(
                        out[it * 128:(it + 1) * 128, esl], ob[:]
                    )

def build_nc(causal: bool = True, reps: int = 1):
    nc = bacc.Bacc(
        "TRN2", target_bir_lowering=False, debug=False, num_devices=N_CORES
    )
    dram = {}
    for name, shape, dt in [
        ("xT", [D, S], F32R),
        ("wq", [D, QCOLS], F32R),
        ("wk", [D, HD], F32R),
        ("wv", [D, HD], F32R),
        ("wo", [QCOLS, D], F32R),
        ("cosq", [HD, S], F32),
        ("sinq", [HD, S], F32),
        ("cosk", [HD, S], F32),
        ("sink", [HD, S], F32),
        ("ones", [128, 128], F32R),
        ("ident", [128, 128], F32R),
    ]:
        dram[name] = nc.dram_tensor(name, shape, dt, kind="ExternalInput").ap()
    out = nc.dram_tensor("out", [S, D], F32, kind="ExternalOutput").ap()

    with tile.TileContext(nc) as tc:
        for _ in range(reps):
            _emit_body(nc, tc, dram, out, causal)

    nc.compile()
    return nc


def get_nc(causal: bool = True):
    if causal not in _BUILD_CACHE:
        _BUILD_CACHE[causal] = build_nc(causal)
    return _BUILD_CACHE[causal]


def prep_in_maps(x, sincos, wq, wk, wv, wo):
    """Host-side shard + layout prep. Returns list of per-core input dicts."""
    x = np.asarray(x, np.float32)
    assert x.shape == (1, S, D)
    xT = np.ascontiguousarray(x[0].T)

    sincos = np.asarray(sincos, np.float32)
    sin = sincos[:S, :HD]
    cos = sincos[:S, HD:]
    sinT = np.ascontiguousarray(sin.T)
    cosT = np.ascontiguousarray(cos.T)
    sin_sgn = sinT.copy()
    sin_sgn[:64] = -sinT[:64]
    scale = np.float32(1.0 / math.sqrt(HD))
    cosq, sinq = cosT * scale, sin_sgn * scale
    cosk, sink = cosT, sin_sgn

    wq = np.asarray(wq, np.float32)
    wk = np.asarray(wk, np.float32)
    wv = np.asarray(wv, np.float32)
    wo = np.asarray(wo, np.float32)

    in_maps = []
    for c in range(N_CORES):
        in_maps.append(
            {
                "xT": xT,
                "wq": np.ascontiguousarray(wq[:, c * QCOLS:(c + 1) * QCOLS]),
                "wk": np.ascontiguousarray(wk[:, c * HD:(c + 1) * HD]),
                "wv": np.ascontiguousarray(wv[:, c * HD:(c + 1) * HD]),
                "wo": np.ascontiguousarray(wo[c * QCOLS:(c + 1) * QCOLS, :]),
                "cosq": cosq,
                "sinq": sinq,
                "cosk": cosk,
                "sink": sink,
                "ones": np.ones((128, 128), np.float32),
                "ident": np.eye(128, dtype=np.float32),
            }
        )
    return in_maps


def check_mask(full_causal_mask, start_pos) -> bool:
    """Returns True for causal (tril) mask, False for all-allowed."""
    sp = int(start_pos)
    assert sp == 0, f"kernel specialized for start_pos=0, got {sp}"
    m = np.asarray(full_causal_mask)
    assert m.shape == (1, 1, S, S)
    m = m[0, 0]
    tril = np.tril(np.ones((S, S), dtype=bool))
    if (m == tril).all():
        return True
    if m.all():
        return False
    raise AssertionError("unsupported mask pattern")


def kernel(
    x,
    start_pos,
    sincos,
    full_causal_mask,
    wq,
    wk,
    wv,
    wo,
    cache_k,
    cache_v,
):
    causal = check_mask(full_causal_mask, start_pos)
    # cache_k/cache_v are zero and fully overwritten in the attended region
    # (start_pos=0, seq_len == max_seq_len) — they do not affect the output.
    nc = get_nc(causal)
    in_maps = prep_in_maps(x, sincos, wq, wk, wv, wo)
    res = run_bass_kernel_spmd(nc, in_maps, list(range(N_CORES)))
    acc = res.results[0]["out"].astype(np.float32)
    for c in range(1, N_CORES):
        acc = acc + res.results[c]["out"]
    return acc[np.newaxis]
